# revision 1
# baseline (speedup 1.0000x reference)
"""Causal GQA attention (B=2,T=2048,D=1024,H=16,KV=4) on 8 trn2 cores.

Sharding: core = b*4 + g  (batch b, kv-group g).  Each core computes the
4 query heads of its group for its batch plus the row-parallel partial of
the output projection; the host sums the 4 partials per batch.
"""

import os
import numpy as np
import ml_dtypes

import concourse.bass as bass
import concourse.tile as tile
import concourse.mybir as mybir
from concourse import bacc
from concourse.bass_utils import run_bass_kernel_spmd
from concourse.masks import make_identity

F32 = mybir.dt.float32
BF16 = mybir.dt.bfloat16
AF = mybir.ActivationFunctionType

B, T, C, HEADS, KVH, HD = 2, 2048, 1024, 16, 4, 64
G = HEADS // KVH          # 4 query heads per kv group
DG = G * HD               # 256 columns per group
NCORES = 8
SCALE = 1.0 / 8.0         # 1/sqrt(HD)

_CACHE = {}
LAST_EXEC_NS = None


def _install_trace_hook():
    import sys, types
    try:
        import antenv.axon_hooks  # noqa: F401
        return
    except ImportError:
        pass
    try:
        from trn_agent_boot.trn_boot import _ntff_profile_via_ctypes
        hook = _ntff_profile_via_ctypes('/opt/axon/libaxon_pjrt.so')
    except Exception:
        hook = None
    mod = types.ModuleType('antenv.axon_hooks')
    mod.get_axon_ntff_profile_hook = lambda: hook
    mod.set_axon_ntff_profile_hook = lambda h: None
    sys.modules['antenv.axon_hooks'] = mod


def _build(debug=False):
    nc = bacc.Bacc("TRN2", target_bir_lowering=False, debug=debug)

    xT_d = nc.dram_tensor("xT", [C, T], BF16, kind="ExternalInput")
    sin2t_d = nc.dram_tensor("sin2t", [128, T], F32, kind="ExternalInput")
    cos2t_d = nc.dram_tensor("cos2t", [128, T], F32, kind="ExternalInput")
    maskb_d = nc.dram_tensor("maskb", [16, 128], F32, kind="ExternalInput")
    wq_d = nc.dram_tensor("wq", [C, DG], BF16, kind="ExternalInput")
    wk_d = nc.dram_tensor("wk", [C, HD], BF16, kind="ExternalInput")
    wv_d = nc.dram_tensor("wv", [C, HD], BF16, kind="ExternalInput")
    wo_d = nc.dram_tensor("wo", [DG, C], BF16, kind="ExternalInput")
    rt_d = nc.dram_tensor("rt", [128, 128], BF16, kind="ExternalInput")
    mska_d = nc.dram_tensor("mska", [128, 1024], BF16, kind="ExternalInput")
    mskb2_d = nc.dram_tensor("mskb2", [128, 1024], BF16, kind="ExternalInput")
    y_d = nc.dram_tensor("y", [T, C], F32, kind="ExternalOutput")

    NT = T // 512             # 4 blocks of 512 along t/q
    NKT = T // 128            # 16 k tiles of 128

    with tile.TileContext(nc) as tc:
        with (
            tc.tile_pool(name="persist", bufs=1) as persist,
            tc.tile_pool(name="stage", bufs=3) as stage,
        ):
            with tc.tile_pool(name="early", bufs=1) as early:
                # ---- psum bank reservation + PE heater ----
                # the "sp" tag grabs the low psum banks so attention scores
                # never contend with projection-chain banks
                ps_cm = tc.tile_pool(name="ps", bufs=2, space="PSUM")
                ps = ps_cm.__enter__()

                # ---- small constants ----
                rt_sb = early.tile([128, 128], BF16, tag="rt")
                nc.sync.dma_start(out=rt_sb[:], in_=rt_d[:, :])
                mska = persist.tile([128, 1024], BF16, tag="mska")
                nc.sync.dma_start(out=mska[:], in_=mska_d[:, :])
                mskb2 = persist.tile([128, 1024], BF16, tag="mskb2")
                nc.sync.dma_start(out=mskb2[:], in_=mskb2_d[:, :])
                mb_sb = early.tile([16, 128], F32, tag="mb")
                nc.sync.dma_start(out=mb_sb[:], in_=maskb_d[:, :])

                # pin the "sp" tag to the low psum banks
                heater = ps.tile([128, 1024], F32, tag="sp")
                nc.vector.memset(heater[0:1, 0:8], 0.0)

                id16 = early.tile([16, 16], F32, tag="id16")
                make_identity(nc, id16[:])
                id64b = early.tile([64, 64], BF16, tag="id64b")
                make_identity(nc, id64b[:])
                # e4[:, h, :]: [4, 64] selector picking row h
                e4 = persist.tile([4, 4, 64], BF16, tag="e4")
                nc.gpsimd.memset(e4[:], 0.0)
                nc.gpsimd.affine_select(
                    out=e4[:], in_=e4[:],
                    compare_op=mybir.AluOpType.not_equal,
                    fill=1.0, base=0,
                    pattern=[[-1, 4], [0, 64]],
                    channel_multiplier=1)

                # ---- weights (bf16 from host) ----
                wqbf = early.tile([128, 8, DG], BF16, tag="wqbf")
                wkbf = early.tile([128, 8, HD], BF16, tag="wkbf")
                wvbf = early.tile([128, 8, HD], BF16, tag="wvbf")
                for ct in range(8):
                    cs = slice(ct * 128, (ct + 1) * 128)
                    nc.sync.dma_start(out=wkbf[:, ct, :], in_=wk_d[cs, :])
                    nc.sync.dma_start(out=wvbf[:, ct, :], in_=wv_d[cs, :])
                    nc.sync.dma_start(out=wqbf[:, ct, :], in_=wq_d[cs, :])
                wobf = persist.tile([128, 2, C], BF16, tag="wobf")
                for mi in range(2):
                    nc.sync.dma_start(out=wobf[:, mi, :],
                                      in_=wo_d[mi * 128:(mi + 1) * 128, :])

                sin2t = early.tile([128, T], F32, tag="sin2t")
                nc.sync.dma_start(out=sin2t[:], in_=sin2t_d[:, :])
                cos2t = early.tile([128, T], F32, tag="cos2t")
                nc.sync.dma_start(out=cos2t[:], in_=cos2t_d[:, :])
                xtbf = early.tile([128, 8, T], BF16, tag="xtbf")
                for ct in range(8):
                    for hh in range(2):
                        hs = slice(hh * 1024, (hh + 1) * 1024)
                        nc.sync.dma_start(out=xtbf[:, ct, hs],
                                          in_=xT_d[ct * 128:(ct + 1) * 128, hs])

                # padding mask -> per-k 0/1 column layout [128, NKT]
                kmask01 = early.tile([128, NKT], F32, tag="kmask01")
                with tc.tile_pool(name="pp0", bufs=1, space="PSUM") as pp0:
                    mt = pp0.tile([128, 16], F32, tag="mt")
                    nc.tensor.transpose(mt[:], mb_sb[:], id16[:])
                    nc.vector.tensor_scalar(
                        out=kmask01[:], in0=mt[:], scalar1=0.0, scalar2=None,
                        op0=mybir.AluOpType.is_gt)

                # ---- projections + RoPE (transposed layout) ----
                # qhat_pair[m]: [128, T] rows 0:64 = head 2m, 64:128 = head 2m+1
                qhat_pair = [persist.tile([128, T], BF16, tag=f"qhatp{m}", name=f"qhatp{m}") for m in range(2)]
                qodd = [persist.tile([64, T], BF16, tag=f"qodd{m}", name=f"qodd{m}") for m in range(2)]
                khat = persist.tile([64, T], BF16, tag="khat")
                vtbf = early.tile([64, T], BF16, tag="vtbf")
                vp = persist.tile([128, NKT, HD + 1], BF16, tag="vp")
                nc.vector.memset(vp[:, :, HD:HD + 1], 1.0)

                with (
                    tc.tile_pool(name="pj", bufs=2, space="PSUM") as pj,
                    tc.tile_pool(name="pr", bufs=2, space="PSUM") as pr,
                ):
                    # K first (attention waits on it), then V, then Q
                    for tb in range(NT):
                        ts_ = slice(tb * 512, (tb + 1) * 512)
                        pk = pj.tile([64, 512], F32, tag="pp")
                        for ct in range(8):
                            nc.tensor.matmul(pk[:], wkbf[:, ct, :], xtbf[:, ct, ts_],
                                             start=(ct == 0), stop=(ct == 7))
                        kb_bf = stage.tile([64, 512], BF16, tag="ktmp")
                        nc.vector.tensor_copy(out=kb_bf[:], in_=pk[:])
                        krot = pr.tile([64, 512], F32, tag="pr")
                        nc.tensor.matmul(krot[:], rt_sb[0:64, 0:64], kb_bf[:],
                                         start=True, stop=True)
                        t1k = stage.tile([64, 512], F32, tag="t1k")
                        nc.vector.tensor_mul(t1k[:], pk[:], cos2t[0:64, ts_])
                        t2k = stage.tile([64, 512], F32, tag="t2k")
                        nc.vector.tensor_mul(t2k[:], krot[:], sin2t[0:64, ts_])
                        nc.vector.tensor_add(khat[:, ts_], t1k[:], t2k[:])
                    # V + V' build
                    for tb in range(NT):
                        ts_ = slice(tb * 512, (tb + 1) * 512)
                        pv = pj.tile([64, 512], F32, tag="pp")
                        for ct in range(8):
                            nc.tensor.matmul(pv[:], wvbf[:, ct, :], xtbf[:, ct, ts_],
                                             start=(ct == 0), stop=(ct == 7))
                        nc.vector.tensor_copy(out=vtbf[:, ts_], in_=pv[:])
                        for k4 in range(4):
                            kt = tb * 4 + k4
                            vt_ps = pr.tile([128, 64], BF16, tag="pr")
                            nc.tensor.transpose(
                                vt_ps[:], vtbf[:, kt * 128:(kt + 1) * 128], id64b[:])
                            nc.vector.tensor_copy(out=vp[:, kt, 0:HD], in_=vt_ps[:])
                            nc.gpsimd.tensor_scalar_mul(
                                vp[:, kt, :], vp[:, kt, :], kmask01[:, kt:kt + 1])
                    # Q pairs
                    for m in range(2):
                        for tb in range(NT):
                            ts_ = slice(tb * 512, (tb + 1) * 512)
                            pq = pj.tile([128, 512], F32, tag="pp")
                            for ct in range(8):
                                nc.tensor.matmul(
                                    pq[:], wqbf[:, ct, m * 128:(m + 1) * 128],
                                    xtbf[:, ct, ts_],
                                    start=(ct == 0), stop=(ct == 7))
                            qb_bf = stage.tile([128, 512], BF16, tag="qtmp")
                            nc.vector.tensor_copy(out=qb_bf[:], in_=pq[:])
                            prot = pr.tile([128, 512], F32, tag="pr")
                            nc.tensor.matmul(prot[:], rt_sb[:], qb_bf[:],
                                             start=True, stop=True)
                            t1 = stage.tile([128, 512], F32, tag="t1", bufs=2)
                            nc.vector.tensor_mul(t1[:], pq[:], cos2t[:, ts_])
                            t2 = stage.tile([128, 512], F32, tag="t2", bufs=2)
                            nc.vector.tensor_mul(t2[:], prot[:], sin2t[:, ts_])
                            nc.vector.tensor_add(qhat_pair[m][:, ts_], t1[:], t2[:])
                        nc.sync.dma_start(out=qodd[m][:, :],
                                          in_=qhat_pair[m][64:128, :])

            # ---- fused attention + normalize + output, per q-block ----
            ctxn = [persist.tile([128, T], BF16, tag=f"ctxn{mi}", name=f"ctxn{mi}") for mi in range(2)]
            late_cm = tc.tile_pool(name="late", bufs=1)
            late = late_cm.__enter__()
            ctxu = late.tile([64, 16, 512], F32, tag="ctxu")
            l16 = late.tile([4, NT, 512], F32, tag="l16")
            r16 = late.tile([4, NT, 512], F32, tag="r16")
            rscr = late.tile([4, 512], F32, tag="rscr")
            rbf = late.tile([4, NT, 512], BF16, tag="rbf")
            rcb = late.tile([4, 512], F32, tag="rcb")
            rres = late.tile([4, 512], F32, tag="rres")
            rres_bf = late.tile([4, NT, 512], BF16, tag="rres_bf")
            with (
                tc.tile_pool(name="pc", bufs=2, space="PSUM") as pc,
                tc.tile_pool(name="pb", bufs=1, space="PSUM") as pb,
                tc.tile_pool(name="py", bufs=1, space="PSUM") as py,
            ):
                for qb in range(NT):
                    qs_ = slice(qb * 512, (qb + 1) * 512)
                    kt_last = 4 * qb + 3
                    for h in range(G):
                        m, lo = divmod(h, 2)
                        qrhs = qhat_pair[m] if lo == 0 else qodd[m]
                        u = qb * G + h
                        ctx = pc.tile([65, 512], F32, tag="ctx")
                        for pi in range(2 * (qb + 1)):
                            sp = ps.tile([128, 1024], F32, tag="sp")
                            for half in range(2):
                                kt = 2 * pi + half
                                nc.tensor.matmul(
                                    sp[:, half * 512:(half + 1) * 512],
                                    khat[:, kt * 128:(kt + 1) * 128],
                                    qrhs[0:64, qs_],
                                    start=True, stop=True)
                            pbf = stage.tile([128, 1024], BF16, tag="pbf", bufs=8)
                            nc.scalar.activation(pbf[:], sp[:], AF.Exp,
                                                 bias=0.0, scale=SCALE)
                            if pi == 2 * qb:
                                nc.vector.tensor_mul(pbf[:, 0:512], pbf[:, 0:512],
                                                     mska[:, 0:512])
                                nc.vector.tensor_mul(pbf[:, 512:1024],
                                                     pbf[:, 512:1024],
                                                     mska[:, 512:1024])
                            elif pi == 2 * qb + 1:
                                nc.vector.tensor_mul(pbf[:, 0:512], pbf[:, 0:512],
                                                     mskb2[:, 0:512])
                                nc.vector.tensor_mul(pbf[:, 512:1024],
                                                     pbf[:, 512:1024],
                                                     mskb2[:, 512:1024])
                            for half in range(2):
                                kt = 2 * pi + half
                                nc.tensor.matmul(
                                    ctx[:], vp[:, kt, :],
                                    pbf[:, half * 512:(half + 1) * 512],
                                    start=(kt == 0), stop=(kt == kt_last))
                        # evict unnormalized ctx + the l row (partition 64)
                        nc.vector.tensor_copy(out=ctxu[:, u, :], in_=ctx[0:64, :])
                        lrow = stage.tile([65, 512], F32, tag="lrow", bufs=2)
                        nc.vector.tensor_copy(out=lrow[64:65, :], in_=ctx[64:65, :])
                        nc.sync.dma_start(out=l16[h:h + 1, qb, :],
                                          in_=lrow[64:65, :])
                    # normalize this q-block: 1/l with bf16+residual split
                    nc.vector.reciprocal_approx_accurate(
                        r16[:, qb, :], l16[:, qb, :], rscr[:])
                    nc.vector.tensor_copy(out=rbf[:, qb, :], in_=r16[:, qb, :])
                    nc.vector.tensor_copy(out=rcb[:], in_=rbf[:, qb, :])
                    nc.vector.tensor_sub(rres[:], r16[:, qb, :], rcb[:])
                    nc.vector.tensor_copy(out=rres_bf[:, qb, :], in_=rres[:])
                    for h in range(G):
                        m, lo = divmod(h, 2)
                        u = qb * G + h
                        bc = pb.tile([64, 512], F32, tag="bc")
                        nc.tensor.matmul(bc[:], e4[:, h, :], rbf[:, qb, :],
                                         start=True, stop=False)
                        nc.tensor.matmul(bc[:], e4[:, h, :], rres_bf[:, qb, :],
                                         start=False, stop=True)
                        cn = stage.tile([64, 512], BF16, tag="cn")
                        nc.vector.tensor_mul(cn[:], ctxu[:, u, :], bc[:])
                        nc.sync.dma_start(
                            out=ctxn[m][lo * 64:(lo + 1) * 64, qs_], in_=cn[:])
                    # output projection for this q-block
                    for ti in range(4):
                        tt = qb * 4 + ti
                        ysb = stage.tile([128, C], F32, tag="ysb", bufs=2)
                        for eb in range(2):
                            yp = py.tile([128, 512], F32, tag="yp")
                            for mi in range(2):
                                nc.tensor.matmul(
                                    yp[:], ctxn[mi][:, tt * 128:(tt + 1) * 128],
                                    wobf[:, mi, eb * 512:(eb + 1) * 512],
                                    start=(mi == 0), stop=(mi == 1))
                            nc.vector.tensor_copy(
                                out=ysb[:, eb * 512:(eb + 1) * 512], in_=yp[:])
                        nc.sync.dma_start(
                            out=y_d[tt * 128:(tt + 1) * 128, :], in_=ysb[:])

            late_cm.__exit__(None, None, None)
            ps_cm.__exit__(None, None, None)

    nc.compile()
    return nc


def _host_constants():
    # rotation matrix (lhsT layout): rot = R @ qT with R[2i,2i+1]=-1, R[2i+1,2i]=1
    rt = np.zeros((128, 128), np.float32)
    i = np.arange(0, 128, 2)
    rt[i + 1, i] = -1.0     # lhsT[j, d] = R[d, j]
    rt[i, i + 1] = 1.0
    rt_bf = rt.astype(ml_dtypes.bfloat16)

    f = np.arange(512)[None, :]
    p = np.arange(128)[:, None]

    def mk(o0, o1):
        m0 = (f - p - o0) >= 0
        m1 = (f - p - o1) >= 0
        return np.concatenate([m0, m1], axis=1).astype(ml_dtypes.bfloat16)

    return rt_bf, mk(0, 128), mk(256, 384)


def kernel(x, sin, cos, mask, Wq, Wk, Wv, Wo):
    global LAST_EXEC_NS
    if "nc" not in _CACHE:
        _CACHE["nc"] = _build()
    nc = _CACHE["nc"]

    x = np.asarray(x, np.float32)
    sin = np.asarray(sin, np.float32)
    cos = np.asarray(cos, np.float32)
    mask = np.asarray(mask, np.float32)
    Wq, Wk, Wv, Wo = (np.asarray(w, np.float32) for w in (Wq, Wk, Wv, Wo))

    sinT = np.ascontiguousarray(sin.T)            # [64, T]
    sin2t = np.concatenate([sinT, sinT], axis=0)  # [128, T]
    cosT = np.ascontiguousarray(cos.T)
    cos2t = np.concatenate([cosT, cosT], axis=0)
    rt_bf, mska, mskb2 = _host_constants()

    in_maps = []
    for core in range(NCORES):
        b, g = divmod(core, KVH)
        in_maps.append({
            "xT": np.ascontiguousarray(x[b].T).astype(ml_dtypes.bfloat16),
            "sin2t": sin2t,
            "cos2t": cos2t,
            "maskb": np.ascontiguousarray(mask[b, 0].reshape(16, 128)),
            "wq": np.ascontiguousarray(Wq[:, g * DG:(g + 1) * DG]).astype(ml_dtypes.bfloat16),
            "wk": np.ascontiguousarray(Wk[:, g * HD:(g + 1) * HD]).astype(ml_dtypes.bfloat16),
            "wv": np.ascontiguousarray(Wv[:, g * HD:(g + 1) * HD]).astype(ml_dtypes.bfloat16),
            "wo": np.ascontiguousarray(Wo[g * DG:(g + 1) * DG, :]).astype(ml_dtypes.bfloat16),
            "rt": rt_bf,
            "mska": mska,
            "mskb2": mskb2,
        })

    trace = os.environ.get("KERNEL_TRACE", "0") == "1"
    if trace:
        _install_trace_hook()
    res = run_bass_kernel_spmd(nc, in_maps, core_ids=list(range(NCORES)),
                               trace=trace)
    LAST_EXEC_NS = res.exec_time_ns

    y = np.zeros((B, T, C), np.float32)
    for core in range(NCORES):
        b = core // KVH
        y[b] += res.results[core]["y"]
    return y

